# revision 10
# baseline (speedup 1.0000x reference)
"""Multi-head attention (B=2, T=2048, D=1024, H=16, Dh=64) on 8 TRN2 NeuronCores.

Sharding: core c = 4*b + g  ->  batch b in {0,1}, head-group g in {0..3}
(4 heads per core: data parallel on batch, tensor parallel on heads).
Each core computes, for its batch element and its 4 heads:

  Q.T/K.T = Wq/k_shard.T @ x.T + b      [256, 2048]  (head-dim on partitions)
  V'      = x @ Wv_interleaved + b      [2048, 260]  ([V_h | 1] per head)
  per head pair p, per 512-wide i-chunk:
    S.T   = K_h Q_h.T                   (two K=64 matmuls on disjoint PE
                                         row groups -> run concurrently)
    P.T   = exp(S.T / 8)                (no max-subtraction: |S|/8 <~ 6)
    acc   = [V_h | 1].T @ P.T           [65, 512]  row 64 = softmax denom
    attnT = acc[:64] * (1/acc[64])
  partial = attnT.T @ Wout_shard        [2048, 1024]

The partial sum over the 4 head groups plus b_out is done on the host
("all-reduce after out_proj"), as is the batch unshard.

Matmuls run in bf16 (1 pass/row on the PE; fp32 PSUM accumulate).
The kernel is one flat software pipeline over the 8 attention groups in
ic-major order ((0,0),(1,0),(0,1),...): the PE is the bottleneck engine,
so projection work (Q/K at N<=1024, V just-in-time, out-proj tiles as
their at-chunks complete) is spread as filler across the j-loops to keep
its queue full.  Input DMAs are split/ordered so the first matmul can
start ~3us in; each group's softmax normalization runs at the end of its
own j-loop, hidden under the next group's first exp.
"""

import os
import numpy as np
import ml_dtypes

bf16 = ml_dtypes.bfloat16

B, T, D = 2, 2048, 1024
H, DH = 16, 64
NCORES, GROUPS = 8, 4
HPC = H // GROUPS        # 4 heads per core
F = HPC * DH             # 256 features per core
FT = F // 128            # 2 feature tiles / head pairs
KTN = D // 128           # 8 contraction tiles
TT = T // 128            # 16 token tiles
NCH = 512                # attention i-chunk width
VW = DH + 1              # 65: V plus ones column
VF = HPC * VW            # 260: interleaved [V_h | 1] x 4 heads

_prog = None
LAST_RESULT = None


def _build():
    from contextlib import ExitStack

    import concourse.mybir as mybir
    import concourse.tile as tile
    from concourse import bacc
    from concourse.bass import ts

    f32 = mybir.dt.float32
    f16 = mybir.dt.bfloat16
    Exp = mybir.ActivationFunctionType.Exp

    nc = bacc.Bacc()
    xT = nc.dram_tensor("xT", [D, T], f16, kind="ExternalInput")
    wq = nc.dram_tensor("wq", [D, F], f16, kind="ExternalInput")
    wk = nc.dram_tensor("wk", [D, F], f16, kind="ExternalInput")
    # wv/bv come pre-interleaved from the host: column h*65+64 is a zero
    # weight column whose bias is 1.0, producing the [V_h | 1] layout that
    # supplies the softmax-denominator row of the PV matmul for free.
    wv = nc.dram_tensor("wv", [D, VF], f16, kind="ExternalInput")
    bq = nc.dram_tensor("bq", [F, 1], f32, kind="ExternalInput")
    bk = nc.dram_tensor("bk", [F, 1], f32, kind="ExternalInput")
    bv = nc.dram_tensor("bv", [1, VF], f16, kind="ExternalInput")
    wo = nc.dram_tensor("wo", [F, D], f16, kind="ExternalInput")
    out = nc.dram_tensor("out", [T, D], f16, kind="ExternalOutput")

    with ExitStack() as ctx:
        tc = ctx.enter_context(tile.TileContext(nc))
        pers = ctx.enter_context(tc.tile_pool(name="pers", bufs=1))
        ptp = ctx.enter_context(tc.tile_pool(name="ptp", bufs=2))
        osb = ctx.enter_context(tc.tile_pool(name="osb", bufs=2))
        msc = ctx.enter_context(tc.tile_pool(name="msc", bufs=2))
        # double-buffered 1-bank PSUM slot shared by all filler work (Q/K/V/
        # out projections + norm broadcast); matmul N<=512 (one bank) is an
        # ISA limit, so every projection works in 512-wide chunks
        psf = ctx.enter_context(tc.tile_pool(name="psf", bufs=2, space="PSUM"))
        pss = ctx.enter_context(tc.tile_pool(name="pss", bufs=2, space="PSUM"))
        pso = ctx.enter_context(tc.tile_pool(name="pso", bufs=1, space="PSUM"))

        xt = pers.tile([128, KTN, T], f16, tag="xt")
        wqs = pers.tile([128, KTN, F], f16, tag="wqs")
        wks = pers.tile([128, KTN, F], f16, tag="wks")
        wvs = pers.tile([128, KTN, VF], f16, tag="wvs")
        bqc = pers.tile([128, FT, 1], f32, tag="bqc")
        bkc = pers.tile([128, FT, 1], f32, tag="bkc")
        bvr = pers.tile([1, VF], f16, tag="bvr")
        ones_f = pers.tile([1, 128], f32, tag="ones_f")
        ones16 = pers.tile([1, 128], f16, tag="ones16")
        wos = pers.tile([128, FT, D], f16, tag="wos")
        qt = pers.tile([128, FT, T], f16, tag="qt")
        kt = pers.tile([128, FT, T], f16, tag="kt")
        vs = pers.tile([128, TT, VF], f16, tag="vs")
        at = pers.tile([128, FT, T], f16, tag="at")

        # ISA memset can't target bf16; memset f32 then copy-convert
        nc.vector.memset(ones_f[:], 1.0)
        nc.vector.tensor_copy(ones16[:], ones_f[:])

        # ---- input DMAs, ordered so first-consumed data arrives first;
        # subtile deps let the prologue matmuls pipeline with these, so
        # attention starts ~6us in instead of waiting for all inputs.
        XCH = 512
        for k in range(KTN):
            nc.sync.dma_start(xt[:, k, 0:XCH], xT[ts(k, 128), 0:XCH])
        for k in range(KTN):
            nc.sync.dma_start(wks[:, k, :], wk[ts(k, 128), :])
        for ft in range(FT):
            nc.sync.dma_start(bkc[:, ft, :], bk[ts(ft, 128), :])
        for k in range(KTN):
            nc.sync.dma_start(wqs[:, k, :], wq[ts(k, 128), :])
        for ft in range(FT):
            nc.sync.dma_start(bqc[:, ft, :], bq[ts(ft, 128), :])
        for k in range(KTN):
            nc.sync.dma_start(wvs[:, k, :], wv[ts(k, 128), :])
        nc.sync.dma_start(bvr[:], bv[:])
        for k in range(KTN):
            nc.sync.dma_start(xt[:, k, XCH:2 * XCH], xT[ts(k, 128), XCH:2 * XCH])
        for k in range(KTN):
            nc.sync.dma_start(xt[:, k, 2 * XCH:T], xT[ts(k, 128), 2 * XCH:T])
        for ft in range(FT):
            nc.sync.dma_start(wos[:, ft, :], wo[ts(ft, 128), :])

        # ---- deferred work units (emitted inside attention j-loops) ----
        def qk_chunk(wsb, bcol, dst, ft, c):
            """Project 512 columns of Q.T or K.T (chunk c)."""
            def go():
                ps = psf.tile([128, NCH], f32, tag="fill", name="ps")
                for k in range(KTN):
                    nc.tensor.matmul(
                        ps[:],
                        wsb[:, k, ts(ft, 128)],
                        xt[:, k, ts(c, NCH)],
                        start=(k == 0), stop=(k == KTN - 1),
                    )
                nc.vector.tensor_scalar_add(
                    dst[:, ft, ts(c, NCH)], ps[:], bcol[:, ft, :]
                )
            return go

        def v_tile(t):
            def go():
                pv = psf.tile([128, VF], f32, tag="fill", name="pv",
                              padded_shape=[128, NCH])
                for k in range(KTN):
                    nc.tensor.matmul(
                        pv[:], xt[:, k, ts(t, 128)], wvs[:, k, :],
                        start=(k == 0), stop=False,
                    )
                # bias via ones-row (also writes the denominator 1.0 cols)
                nc.tensor.matmul(
                    pv[:], ones16[:, 0:128], bvr[:], start=False, stop=True
                )
                nc.vector.tensor_copy(vs[:, t, :], pv[:])
            return go

        def outproj_tile(t):
            def go():
                ob = osb.tile([128, D], f16, tag="ob", name="ob")
                for c in range(D // NCH):
                    pp = psf.tile([128, NCH], f32, tag="fill", name="pp")
                    for ft in range(FT):
                        nc.tensor.matmul(
                            pp[:],
                            at[:, ft, ts(t, 128)],
                            wos[:, ft, ts(c, NCH)],
                            start=(ft == 0), stop=(ft == FT - 1),
                        )
                    nc.vector.tensor_copy(ob[:, ts(c, NCH)], pp[:])
                nc.sync.dma_start(out[ts(t, 128), :], ob[:])
            return go

        def norm(p, ic, accs):
            """Softmax normalization for group (p, ic): attnT = num/denom."""
            for hh in range(2):
                acc = accs[hh]
                # custom-DVE ops drop the partition base offset; stage
                # the denominator row to SBUF partition 0 first
                dn = msc.tile([1, NCH], f32, tag="dn", bufs=2)
                nc.vector.tensor_copy(dn[:], acc[DH: DH + 1, :])
                rc = msc.tile([1, NCH], f32, tag="rc", bufs=2)
                nc.vector.reciprocal_approx_fast(rc[:], dn[:])
                rcr = msc.tile([1, NCH], f16, tag="rcr", bufs=2)
                nc.vector.tensor_copy(rcr[:], rc[:])  # round to bf16
                pb = psf.tile([64, NCH], f32, tag="fill", name="pb")
                nc.tensor.matmul(
                    pb[:], ones16[:, 0:64], rcr[:], start=True, stop=True
                )
                bsb = msc.tile([64, NCH], f32, tag="bsb")
                nc.vector.tensor_copy(bsb[:], pb[:])
                dst_sl = ts(ic, NCH)
                if hh == 0:
                    nc.vector.tensor_mul(
                        at[0:DH, p, dst_sl], acc[0:DH, :], bsb[:]
                    )
                else:
                    # DVE lanes can't shift partitions; bounce via DMA
                    tmp = msc.tile([DH, NCH], f16, tag="tmp", bufs=2)
                    nc.vector.tensor_mul(tmp[:], acc[0:DH, :], bsb[:])
                    nc.sync.dma_start(at[64:128, p, dst_sl], tmp[:])

        def make_scores(p, ic):
            def scores(j):
                # disjoint PE row groups (partitions 0-63 / 64-127): the two
                # K=64 matmuls execute concurrently
                sc = pss.tile([128, 2 * NCH], f32, tag="sc", name="sc")
                for hh in range(2):
                    nc.tensor.matmul(
                        sc[:, ts(hh, NCH)],
                        kt[hh * 64: hh * 64 + DH, p, ts(j, 128)],
                        qt[hh * 64: hh * 64 + DH, p, ts(ic, NCH)],
                        start=True, stop=True,
                    )
                return sc
            return scores

        # p-major group order: group 0 is saturated by the just-in-time V
        # projection, so spread K-ft1/Q fillers over groups 1-3; out-proj
        # chunk c is complete after group 4+c, so its tiles fill 5-7.
        seq = [(p, ic) for p in range(FT) for ic in range(T // NCH)]
        scores_of = {g: make_scores(*g) for g in seq}

        # filler schedule: extras[(gi, j)] = list of thunks
        extras = {}
        def add(gi, j, th):
            extras.setdefault((gi, j), []).append(th)

        for j in range(TT - 1):                     # g0: V proj just-in-time
            add(0, j, v_tile(j + 1))
        add(0, 2, qk_chunk(wks, bkc, kt, 0, 1))              # K ft0, rest
        add(0, 6, qk_chunk(wks, bkc, kt, 0, 2))
        add(0, 10, qk_chunk(wks, bkc, kt, 0, 3))
        add(0, 13, qk_chunk(wqs, bqc, qt, 0, 1))             # Q ft0 ic1 (g1)
        add(1, 2, qk_chunk(wks, bkc, kt, 1, 0))              # K ft1 (for g4)
        add(1, 6, qk_chunk(wks, bkc, kt, 1, 1))
        add(1, 9, qk_chunk(wqs, bqc, qt, 0, 2))              # Q ft0 ic2 (g2)
        add(1, 12, qk_chunk(wqs, bqc, qt, 0, 3))             # Q ft0 ic3 (g3)
        add(2, 3, qk_chunk(wks, bkc, kt, 1, 2))              # K ft1 rest
        add(2, 7, qk_chunk(wks, bkc, kt, 1, 3))
        add(2, 11, qk_chunk(wqs, bqc, qt, 1, 0))             # Q ft1 ic0 (g4)
        add(3, 3, qk_chunk(wqs, bqc, qt, 1, 1))              # Q ft1 ic1 (g5)
        add(3, 7, qk_chunk(wqs, bqc, qt, 1, 2))              # Q ft1 ic2 (g6)
        add(3, 11, qk_chunk(wqs, bqc, qt, 1, 3))             # Q ft1 ic3 (g7)
        add(5, 1, outproj_tile(0))
        add(5, 5, outproj_tile(1))
        add(5, 9, outproj_tile(2))
        add(5, 13, outproj_tile(3))
        add(6, 1, outproj_tile(4))
        add(6, 5, outproj_tile(5))
        add(6, 9, outproj_tile(6))
        add(6, 13, outproj_tile(7))
        add(7, 1, outproj_tile(8))
        add(7, 5, outproj_tile(9))
        add(7, 9, outproj_tile(10))
        add(7, 13, outproj_tile(11))

        # ---- prologue: just enough projection for the first group ----
        qk_chunk(wks, bkc, kt, 0, 0)()
        qk_chunk(wqs, bqc, qt, 0, 0)()
        v_tile(0)()

        # ---- flat attention pipeline over all 8 groups ----
        sc_cur = scores_of[seq[0]](0)
        for gi, (p, ic) in enumerate(seq):
            acc0 = pso.tile([VW, NCH], f32, tag="acc0", name="acc0")
            acc1 = pso.tile([VW, NCH], f32, tag="acc1", name="acc1")
            accs = (acc0, acc1)
            for j in range(TT):
                pe = ptp.tile([128, 2 * NCH], f16, tag="pe", name="pe")
                nc.scalar.activation(pe[:], sc_cur[:], Exp, scale=0.125)
                if j + 1 < TT:
                    sc_cur = scores_of[(p, ic)](j + 1)
                elif gi + 1 < len(seq):
                    sc_cur = scores_of[seq[gi + 1]](0)  # no exp-stream break
                for hh in range(2):
                    nc.tensor.matmul(
                        accs[hh][:, :],
                        vs[:, j, (2 * p + hh) * VW: (2 * p + hh + 1) * VW],
                        pe[:, ts(hh, NCH)],
                        start=(j == 0), stop=(j == TT - 1),
                    )
                for th in extras.get((gi, j), ()):
                    th()
            # normalize at the end of the group; the acc WAR with the next
            # group's first PV matmul resolves under its first exp call
            norm(p, ic, accs)
        for t in range(12, 16):
            outproj_tile(t)()

    nc.finalize()  # Bacc.compile(): wait legalization, reg alloc, act tables
    return nc


def _get_program():
    global _prog
    if _prog is None:
        _prog = _build()
    return _prog


def kernel(x, W_qkv, b_qkv, W_out, b_out):
    global LAST_RESULT
    from concourse.bass_utils import run_bass_kernel_spmd

    x = np.asarray(x, np.float32)
    W_qkv = np.asarray(W_qkv, np.float32)
    b_qkv = np.asarray(b_qkv, np.float32)
    W_out = np.asarray(W_out, np.float32)
    b_out = np.asarray(b_out, np.float32)

    nc = _get_program()

    in_maps = []
    for c in range(NCORES):
        b, g = divmod(c, GROUPS)
        sl = slice(g * F, (g + 1) * F)
        # interleave Wv/bv with [zero-weight, bias=1] columns at h*65+64
        wv_g = W_qkv[:, 2 * D:3 * D][:, sl]
        bv_g = b_qkv[2 * D:3 * D][sl]
        wv_i = np.zeros((D, VF), bf16)
        bv_i = np.zeros((1, VF), bf16)
        for h in range(HPC):
            wv_i[:, h * VW: h * VW + DH] = wv_g[:, h * DH:(h + 1) * DH]
            bv_i[0, h * VW: h * VW + DH] = bv_g[h * DH:(h + 1) * DH]
            bv_i[0, h * VW + DH] = 1.0
        in_maps.append({
            "xT": np.ascontiguousarray(x[b].T.astype(bf16)),
            "wq": np.ascontiguousarray(W_qkv[:, 0 * D:1 * D][:, sl]).astype(bf16),
            "wk": np.ascontiguousarray(W_qkv[:, 1 * D:2 * D][:, sl]).astype(bf16),
            "wv": wv_i,
            "bq": np.ascontiguousarray(b_qkv[0 * D:1 * D][sl][:, None]),
            "bk": np.ascontiguousarray(b_qkv[1 * D:2 * D][sl][:, None]),
            "bv": bv_i,
            "wo": np.ascontiguousarray(W_out[sl, :]).astype(bf16),
        })

    kw = {}
    if os.environ.get("KERNEL_TRACE") == "1":
        kw["trace"] = True
    res = run_bass_kernel_spmd(nc, in_maps, core_ids=list(range(NCORES)), **kw)
    LAST_RESULT = res

    out = np.empty((B, T, D), np.float32)
    for b in range(B):
        acc = res.results[GROUPS * b]["out"].astype(np.float32)
        for g in range(1, GROUPS):
            acc = acc + res.results[GROUPS * b + g]["out"].astype(np.float32)
        out[b] = acc + b_out
    return out


# revision 13
# speedup vs baseline: 1.2634x; 1.2634x over previous
"""Multi-head attention (B=2, T=2048, D=1024, H=16, Dh=64) on 8 TRN2 NeuronCores.

Sharding: core c = 4*b + g  ->  batch b in {0,1}, head-group g in {0..3}
(4 heads per core: data parallel on batch, tensor parallel on heads).
Each core computes, for its batch element and its 4 heads:

  Q.T/K.T = Wq/k_shard.T @ x.T + b      [256, 2048]  (head-dim on partitions)
  V'      = x @ Wv_interleaved + b      [2048, 260]  ([V_h | 1] per head)
  per head pair p, per 512-wide i-chunk:
    S.T   = K_h Q_h.T                   (two K=64 matmuls on disjoint PE
                                         row groups -> run concurrently)
    P.T   = exp(S.T / 8)                (no max-subtraction: |S|/8 <~ 6)
    acc   = [V_h | 1].T @ P.T           [65, 512]  row 64 = softmax denom
    attnT = acc[:64] * (1/acc[64])
  partial = attnT.T @ Wout_shard        [2048, 1024]

The partial sum over the 4 head groups plus b_out is done on the host
("all-reduce after out_proj"), as is the batch unshard.

Matmuls run in bf16 (1 pass/row on the PE; fp32 PSUM accumulate).
The kernel is one flat software pipeline over the 8 attention groups in
ic-major order ((0,0),(1,0),(0,1),...): the PE is the bottleneck engine,
so projection work (Q/K at N<=1024, V just-in-time, out-proj tiles as
their at-chunks complete) is spread as filler across the j-loops to keep
its queue full.  Input DMAs are split/ordered so the first matmul can
start ~3us in; each group's softmax normalization runs at the end of its
own j-loop, hidden under the next group's first exp.
"""

import os
import numpy as np
import ml_dtypes

bf16 = ml_dtypes.bfloat16

B, T, D = 2, 2048, 1024
H, DH = 16, 64
NCORES, GROUPS = 8, 4
HPC = H // GROUPS        # 4 heads per core
F = HPC * DH             # 256 features per core
FT = F // 128            # 2 feature tiles / head pairs
KTN = D // 128           # 8 contraction tiles
TT = T // 128            # 16 token tiles
NCH = 512                # attention i-chunk width
VW = DH + 1              # 65: V plus ones column
VF = HPC * VW            # 260: interleaved [V_h | 1] x 4 heads

_prog = None
LAST_RESULT = None


def _build():
    from contextlib import ExitStack

    import concourse.mybir as mybir
    import concourse.tile as tile
    from concourse import bacc
    from concourse.bass import ts

    f32 = mybir.dt.float32
    f16 = mybir.dt.bfloat16
    Exp = mybir.ActivationFunctionType.Exp

    nc = bacc.Bacc()
    xT = nc.dram_tensor("xT", [D, T], f16, kind="ExternalInput")
    wq = nc.dram_tensor("wq", [D, F], f16, kind="ExternalInput")
    wk = nc.dram_tensor("wk", [D, F], f16, kind="ExternalInput")
    # wv/bv come pre-interleaved from the host: column h*65+64 is a zero
    # weight column whose bias is 1.0, producing the [V_h | 1] layout that
    # supplies the softmax-denominator row of the PV matmul for free.
    wv = nc.dram_tensor("wv", [D, VF], f16, kind="ExternalInput")
    bq = nc.dram_tensor("bq", [F, 1], f32, kind="ExternalInput")
    bk = nc.dram_tensor("bk", [F, 1], f32, kind="ExternalInput")
    bv = nc.dram_tensor("bv", [1, VF], f16, kind="ExternalInput")
    wo = nc.dram_tensor("wo", [F, D], f16, kind="ExternalInput")
    out = nc.dram_tensor("out", [T, D], f16, kind="ExternalOutput")

    with ExitStack() as ctx:
        tc = ctx.enter_context(tile.TileContext(nc))
        pers = ctx.enter_context(tc.tile_pool(name="pers", bufs=1))
        ptp = ctx.enter_context(tc.tile_pool(name="ptp", bufs=2))
        osb = ctx.enter_context(tc.tile_pool(name="osb", bufs=2))
        msc = ctx.enter_context(tc.tile_pool(name="msc", bufs=2))
        # double-buffered 1-bank PSUM slot shared by all filler work (Q/K/V/
        # out projections + norm broadcast); matmul N<=512 (one bank) is an
        # ISA limit, so every projection works in 512-wide chunks
        psf = ctx.enter_context(tc.tile_pool(name="psf", bufs=2, space="PSUM"))
        pss = ctx.enter_context(tc.tile_pool(name="pss", bufs=2, space="PSUM"))
        pso = ctx.enter_context(tc.tile_pool(name="pso", bufs=1, space="PSUM"))

        xt = pers.tile([128, KTN, T], f16, tag="xt")
        wqs = pers.tile([128, KTN, F], f16, tag="wqs")
        wks = pers.tile([128, KTN, F], f16, tag="wks")
        wvs = pers.tile([128, KTN, VF], f16, tag="wvs")
        bqc = pers.tile([128, FT, 1], f32, tag="bqc")
        bkc = pers.tile([128, FT, 1], f32, tag="bkc")
        bvr = pers.tile([1, VF], f16, tag="bvr")
        ones_f = pers.tile([1, 128], f32, tag="ones_f")
        ones16 = pers.tile([1, 128], f16, tag="ones16")
        wos = pers.tile([128, FT, D], f16, tag="wos")
        qt = pers.tile([128, FT, T], f16, tag="qt")
        kt = pers.tile([128, FT, T], f16, tag="kt")
        vs = pers.tile([128, TT, VF], f16, tag="vs")
        at = pers.tile([128, FT, T], f16, tag="at")

        # ISA memset can't target bf16; memset f32 then copy-convert
        nc.vector.memset(ones_f[:], 1.0)
        nc.vector.tensor_copy(ones16[:], ones_f[:])

        # ---- input DMAs: a few large coalesced transfers (issuing a DMA
        # costs ~650ns of sync-engine time, so 63 small ones serialize the
        # head), ordered so first-consumed data arrives first
        def seg(dram, lo, hi, sym="(k p) c -> p k c"):
            return dram[:, lo:hi].rearrange(sym, p=128)

        XCH = 512
        nc.sync.dma_start(xt[:, :, 0:XCH], seg(xT, 0, XCH))
        nc.sync.dma_start(wks[:, :, :], seg(wk, 0, F))
        nc.sync.dma_start(bkc[:, :, :], bk.rearrange("(ft p) o -> p ft o", p=128))
        nc.sync.dma_start(wqs[:, :, :], seg(wq, 0, F))
        nc.sync.dma_start(bqc[:, :, :], bq.rearrange("(ft p) o -> p ft o", p=128))
        nc.sync.dma_start(wvs[:, :, :], seg(wv, 0, VF))
        nc.sync.dma_start(bvr[:], bv[:])
        nc.sync.dma_start(xt[:, :, XCH:2 * XCH], seg(xT, XCH, 2 * XCH))
        nc.sync.dma_start(xt[:, :, 2 * XCH:T], seg(xT, 2 * XCH, T))
        nc.sync.dma_start(wos[:, :, :], wo.rearrange("(ft p) d -> p ft d", p=128))

        # ---- deferred work units (emitted inside attention j-loops) ----
        def qk_chunk(wsb, bcol, dst, ft, c):
            """Project 512 columns of Q.T or K.T (chunk c)."""
            def go():
                ps = psf.tile([128, NCH], f32, tag="fill", name="ps")
                for k in range(KTN):
                    nc.tensor.matmul(
                        ps[:],
                        wsb[:, k, ts(ft, 128)],
                        xt[:, k, ts(c, NCH)],
                        start=(k == 0), stop=(k == KTN - 1),
                    )
                nc.vector.tensor_scalar_add(
                    dst[:, ft, ts(c, NCH)], ps[:], bcol[:, ft, :]
                )
            return go

        def v_tile(t):
            def go():
                pv = psf.tile([128, VF], f32, tag="fill", name="pv",
                              padded_shape=[128, NCH])
                for k in range(KTN):
                    nc.tensor.matmul(
                        pv[:], xt[:, k, ts(t, 128)], wvs[:, k, :],
                        start=(k == 0), stop=False,
                    )
                # bias via ones-row (also writes the denominator 1.0 cols)
                nc.tensor.matmul(
                    pv[:], ones16[:, 0:128], bvr[:], start=False, stop=True
                )
                nc.vector.tensor_copy(vs[:, t, :], pv[:])
            return go

        def outproj_tile(t):
            def go():
                ob = osb.tile([128, D], f16, tag="ob", name="ob")
                for c in range(D // NCH):
                    pp = psf.tile([128, NCH], f32, tag="fill", name="pp")
                    for ft in range(FT):
                        nc.tensor.matmul(
                            pp[:],
                            at[:, ft, ts(t, 128)],
                            wos[:, ft, ts(c, NCH)],
                            start=(ft == 0), stop=(ft == FT - 1),
                        )
                    nc.vector.tensor_copy(ob[:, ts(c, NCH)], pp[:])
                nc.sync.dma_start(out[ts(t, 128), :], ob[:])
            return go

        def norm(p, ic, accs):
            """Softmax normalization for group (p, ic): attnT = num/denom."""
            for hh in range(2):
                acc = accs[hh]
                # custom-DVE ops drop the partition base offset; stage
                # the denominator row to SBUF partition 0 first
                dn = msc.tile([1, NCH], f32, tag="dn", bufs=2)
                nc.vector.tensor_copy(dn[:], acc[DH: DH + 1, :])
                rc = msc.tile([1, NCH], f32, tag="rc", bufs=2)
                nc.vector.reciprocal_approx_fast(rc[:], dn[:])
                rcr = msc.tile([1, NCH], f16, tag="rcr", bufs=2)
                nc.vector.tensor_copy(rcr[:], rc[:])  # round to bf16
                pb = psf.tile([64, NCH], f32, tag="fill", name="pb")
                nc.tensor.matmul(
                    pb[:], ones16[:, 0:64], rcr[:], start=True, stop=True
                )
                bsb = msc.tile([64, NCH], f32, tag="bsb")
                nc.vector.tensor_copy(bsb[:], pb[:])
                dst_sl = ts(ic, NCH)
                if hh == 0:
                    nc.vector.tensor_mul(
                        at[0:DH, p, dst_sl], acc[0:DH, :], bsb[:]
                    )
                else:
                    # DVE lanes can't shift partitions; bounce via DMA
                    tmp = msc.tile([DH, NCH], f16, tag="tmp", bufs=2)
                    nc.vector.tensor_mul(tmp[:], acc[0:DH, :], bsb[:])
                    nc.sync.dma_start(at[64:128, p, dst_sl], tmp[:])

        def make_scores(p, ic):
            def scores(j):
                # disjoint PE row groups (partitions 0-63 / 64-127): the two
                # K=64 matmuls execute concurrently
                sc = pss.tile([128, 2 * NCH], f32, tag="sc", name="sc")
                for hh in range(2):
                    nc.tensor.matmul(
                        sc[:, ts(hh, NCH)],
                        kt[hh * 64: hh * 64 + DH, p, ts(j, 128)],
                        qt[hh * 64: hh * 64 + DH, p, ts(ic, NCH)],
                        start=True, stop=True,
                    )
                return sc
            return scores

        # p-major group order: group 0 is saturated by the just-in-time V
        # projection, so spread K-ft1/Q fillers over groups 1-3; out-proj
        # chunk c is complete after group 4+c, so its tiles fill 5-7.
        seq = [(p, ic) for p in range(FT) for ic in range(T // NCH)]
        scores_of = {g: make_scores(*g) for g in seq}

        # filler schedule: extras[(gi, j)] = list of thunks
        extras = {}
        def add(gi, j, th):
            extras.setdefault((gi, j), []).append(th)

        # fillers at j=0 (or j=15 of the prior group) cover the ~1us window
        # where the next group's first PV matmul waits on the previous
        # group's normalization reads of the PSUM accumulators
        for j in range(TT - 1):                     # g0: V proj just-in-time
            add(0, j, v_tile(j + 1))
        add(0, 2, qk_chunk(wks, bkc, kt, 0, 1))              # K ft0, rest
        add(0, 6, qk_chunk(wks, bkc, kt, 0, 2))
        add(0, 10, qk_chunk(wks, bkc, kt, 0, 3))
        add(0, 13, qk_chunk(wqs, bqc, qt, 0, 1))             # Q ft0 ic1 (g1)
        add(1, 0, qk_chunk(wks, bkc, kt, 1, 0))              # K ft1 (for g4)
        add(1, 5, qk_chunk(wks, bkc, kt, 1, 1))
        add(1, 9, qk_chunk(wqs, bqc, qt, 0, 2))              # Q ft0 ic2 (g2)
        add(1, 13, qk_chunk(wqs, bqc, qt, 0, 3))             # Q ft0 ic3 (g3)
        add(2, 0, qk_chunk(wks, bkc, kt, 1, 2))              # K ft1 rest
        add(2, 5, qk_chunk(wks, bkc, kt, 1, 3))
        add(2, 10, qk_chunk(wqs, bqc, qt, 1, 0))             # Q ft1 ic0 (g4)
        add(3, 0, qk_chunk(wqs, bqc, qt, 1, 1))              # Q ft1 ic1 (g5)
        add(4, 0, qk_chunk(wqs, bqc, qt, 1, 2))              # Q ft1 ic2 (g6)
        add(4, 15, qk_chunk(wqs, bqc, qt, 1, 3))             # Q ft1 ic3 (g7)
        add(5, 1, outproj_tile(0))
        add(5, 5, outproj_tile(1))
        add(5, 9, outproj_tile(2))
        add(5, 15, outproj_tile(3))
        add(6, 1, outproj_tile(4))
        add(6, 5, outproj_tile(5))
        add(6, 9, outproj_tile(6))
        add(6, 15, outproj_tile(7))
        add(7, 1, outproj_tile(8))
        add(7, 5, outproj_tile(9))
        add(7, 9, outproj_tile(10))
        add(7, 15, outproj_tile(11))

        # ---- prologue: just enough projection for the first group ----
        qk_chunk(wks, bkc, kt, 0, 0)()
        qk_chunk(wqs, bqc, qt, 0, 0)()
        v_tile(0)()

        # ---- flat attention pipeline over all 8 groups ----
        sc_cur = scores_of[seq[0]](0)
        for gi, (p, ic) in enumerate(seq):
            acc0 = pso.tile([VW, NCH], f32, tag="acc0", name="acc0")
            acc1 = pso.tile([VW, NCH], f32, tag="acc1", name="acc1")
            accs = (acc0, acc1)
            for j in range(TT):
                pe = ptp.tile([128, 2 * NCH], f16, tag="pe", name="pe")
                nc.scalar.activation(pe[:], sc_cur[:], Exp, scale=0.125)
                if j + 1 < TT:
                    sc_cur = scores_of[(p, ic)](j + 1)
                elif gi + 1 < len(seq):
                    sc_cur = scores_of[seq[gi + 1]](0)  # no exp-stream break
                # fillers BEFORE the PV matmuls: PV waits on exp[j] anyway,
                # so the filler hides that latency (and boundary stalls)
                for th in extras.get((gi, j), ()):
                    th()
                for hh in range(2):
                    nc.tensor.matmul(
                        accs[hh][:, :],
                        vs[:, j, (2 * p + hh) * VW: (2 * p + hh + 1) * VW],
                        pe[:, ts(hh, NCH)],
                        start=(j == 0), stop=(j == TT - 1),
                    )
            # normalize at the end of the group; the acc WAR with the next
            # group's first PV matmul resolves under its first exp call
            norm(p, ic, accs)
        for t in range(12, 16):
            outproj_tile(t)()

    nc.finalize()  # Bacc.compile(): wait legalization, reg alloc, act tables
    return nc


def _get_program():
    global _prog
    if _prog is None:
        _prog = _build()
    return _prog


def kernel(x, W_qkv, b_qkv, W_out, b_out):
    global LAST_RESULT
    from concourse.bass_utils import run_bass_kernel_spmd

    x = np.asarray(x, np.float32)
    W_qkv = np.asarray(W_qkv, np.float32)
    b_qkv = np.asarray(b_qkv, np.float32)
    W_out = np.asarray(W_out, np.float32)
    b_out = np.asarray(b_out, np.float32)

    nc = _get_program()

    in_maps = []
    for c in range(NCORES):
        b, g = divmod(c, GROUPS)
        sl = slice(g * F, (g + 1) * F)
        # interleave Wv/bv with [zero-weight, bias=1] columns at h*65+64
        wv_g = W_qkv[:, 2 * D:3 * D][:, sl]
        bv_g = b_qkv[2 * D:3 * D][sl]
        wv_i = np.zeros((D, VF), bf16)
        bv_i = np.zeros((1, VF), bf16)
        for h in range(HPC):
            wv_i[:, h * VW: h * VW + DH] = wv_g[:, h * DH:(h + 1) * DH]
            bv_i[0, h * VW: h * VW + DH] = bv_g[h * DH:(h + 1) * DH]
            bv_i[0, h * VW + DH] = 1.0
        in_maps.append({
            "xT": np.ascontiguousarray(x[b].T.astype(bf16)),
            "wq": np.ascontiguousarray(W_qkv[:, 0 * D:1 * D][:, sl]).astype(bf16),
            "wk": np.ascontiguousarray(W_qkv[:, 1 * D:2 * D][:, sl]).astype(bf16),
            "wv": wv_i,
            "bq": np.ascontiguousarray(b_qkv[0 * D:1 * D][sl][:, None]),
            "bk": np.ascontiguousarray(b_qkv[1 * D:2 * D][sl][:, None]),
            "bv": bv_i,
            "wo": np.ascontiguousarray(W_out[sl, :]).astype(bf16),
        })

    kw = {}
    if os.environ.get("KERNEL_TRACE") == "1":
        kw["trace"] = True
    res = run_bass_kernel_spmd(nc, in_maps, core_ids=list(range(NCORES)), **kw)
    LAST_RESULT = res

    out = np.empty((B, T, D), np.float32)
    for b in range(B):
        acc = res.results[GROUPS * b]["out"].astype(np.float32)
        for g in range(1, GROUPS):
            acc = acc + res.results[GROUPS * b + g]["out"].astype(np.float32)
        out[b] = acc + b_out
    return out
